# revision 2
# baseline (speedup 1.0000x reference)
"""DistMaps production kernel (v7 pipeline): valid-point specialization + paired strips + trimmed setup.

Same math as v3 (exact-squares K=3 matmul, bf16 sqrt-space maps, Pool
tt-pow, Act tanh-only tail). New: kernel is built for the host-observed
set of valid points (invalid click slots are skipped entirely); strips
are computed two-per-PSUM-tile so copy and pow run once per pair.
build(valid=(tuple_m0, tuple_m1)) with point indices per map.
"""
from contextlib import ExitStack

import numpy as np

import concourse.bass as bass
import concourse.tile as tile
from concourse import bacc, mybir

F32 = mybir.dt.float32
BF16 = mybir.dt.bfloat16
I32 = mybir.dt.int32
AF = mybir.ActivationFunctionType
OP = mybir.AluOpType

B = 8
H = W = 512
NPTS = 24
NPM = 12
NT = 4
WS = 48
CMARG = WS // 2
BIG = 1.0e6
SQBIG = 1000.0


def make_consts():
    parange = np.tile(np.arange(128, dtype=np.float32)[None, :], (NPTS, 1))
    parange128 = parange + 128.0
    w48 = (np.arange(WS, dtype=np.float32))[None, :]
    w96 = np.tile(np.concatenate([w48, w48], 1), (NPTS, 1)).astype(np.float32)
    cbB = np.concatenate([parange, parange128, w96], axis=1)  # [24, 352]
    i0 = np.tile(np.r_[np.ones(WS, np.float32), np.zeros(WS, np.float32)], NPTS)
    i1 = np.tile(np.r_[np.zeros(WS, np.float32), np.ones(WS, np.float32)], NPTS)
    rrc = np.stack([i0, i1])                      # [2, 2304]
    llones = np.ones((1, NPTS * 128), dtype=np.float32)
    return {"cbB": cbB, "rrc": rrc, "llones": llones}


def build(valid=(tuple(range(12)), tuple(range(12, 24))), use_pen=False):
    vm0, vm1 = [list(v) for v in valid]
    nc = bacc.Bacc("TRN2", target_bir_lowering=False, debug=False, num_devices=B)
    coords = nc.dram_tensor("coords", [NPTS, 3], F32, kind="ExternalInput").ap()
    cbB = nc.dram_tensor("cbB", [NPTS, 352], F32, kind="ExternalInput").ap()
    rrc = nc.dram_tensor("rrc", [2, NPTS * 96], F32, kind="ExternalInput").ap()
    llones = nc.dram_tensor("llones", [1, NPTS * 128], F32,
                            kind="ExternalInput").ap()
    y = nc.dram_tensor("y", [2, H, W], F32, kind="ExternalOutput").ap()

    with tile.TileContext(nc) as tc, ExitStack() as ctx:
        pool = ctx.enter_context(tc.tile_pool(name="sb", bufs=1))
        d_pool = ctx.enter_context(tc.tile_pool(name="dmap", bufs=1))
        psum = ctx.enter_context(tc.tile_pool(name="ps", bufs=5, space="PSUM"))
        psum_w = ctx.enter_context(tc.tile_pool(name="psw", bufs=1, space="PSUM"))
        sb_strips = ctx.enter_context(tc.tile_pool(name="sbs", bufs=4))
        out_pool = ctx.enter_context(tc.tile_pool(name="ob", bufs=8))

        # --- input DMAs ---
        coords_sb = pool.tile([NPTS, 3], F32, tag="coords_sb")
        nc.sync.dma_start(coords_sb[:], coords[:])
        cb = pool.tile([NPTS, 352], F32, tag="cb_sb")
        nc.scalar.dma_start(cb[:], cbB[:])
        parange = cb[:, 0:128]
        parange128 = cb[:, 128:256]
        w96 = cb[:, 256:352]
        RR = pool.tile([3, NPTS * 96], F32, tag="RR")
        nc.scalar.dma_start(RR[0:2, :], rrc[:])
        LL = pool.tile([3, NPTS * 128], F32, tag="LL")
        nc.gpsimd.dma_start(LL[2:3, :], llones[:])

        # --- act table prefetch (Tanh) + pow exponent tile (bf16) ---
        dummy = pool.tile([1, 8], F32, tag="dummy")
        nc.gpsimd.memset(dummy[:], 0.0)
        nc.scalar.activation(dummy[:], dummy[:], AF.Tanh, scale=2.0)
        halfs = pool.tile([128, 384], BF16, tag="halfs")
        nc.gpsimd.memset(halfs[:], 0.5)

        dmaps = []
        for m in range(2):
            dm = d_pool.tile([128, NT * W], BF16, tag=f"d{m}")
            dmaps.append(dm)
        nc.gpsimd.memset(dmaps[0][:], SQBIG)

        # --- PE warmup ---
        wps = psum_w.tile([128, 176], F32, tag="warm")
        for _ in range(6):
            nc.tensor.matmul(wps[:], cb[0:1, 0:128], cb[0:1, 0:176],
                             start=True, stop=True)

        # --- geometry (DVE) ---
        r = coords_sb[:, 0:1]
        c = coords_sb[:, 1:2]
        g = pool.tile([NPTS, 8], F32, tag="geo")
        pen = g[:, 0:1]
        # (pen written only when use_pen)
        t0f = g[:, 1:2]
        csf = g[:, 2:3]
        csb = g[:, 3:4]
        rr0 = g[:, 4:5]
        bc = g[:, 5:6]
        yrow = g[:, 6:7]

        v = nc.vector
        if use_pen:
            v.tensor_tensor(out=pen, in0=r, in1=c, op=OP.max)
            v.tensor_scalar(out=pen, in0=pen, scalar1=0.0, scalar2=BIG,
                            op0=OP.is_lt, op1=OP.mult)
        v.tensor_scalar(out=yrow, in0=r, scalar1=-23.0, scalar2=1.0 / 128.0,
                        op0=OP.add, op1=OP.mult)
        v.tensor_scalar(out=t0f, in0=yrow, scalar1=1.0, scalar2=None, op0=OP.is_ge)
        v.scalar_tensor_tensor(out=t0f, in0=yrow, scalar=2.0, in1=t0f,
                               op0=OP.is_ge, op1=OP.add)
        v.tensor_scalar(out=csf, in0=c, scalar1=float(-CMARG), scalar2=0.0,
                        op0=OP.add, op1=OP.max)
        v.tensor_scalar(out=csf, in0=csf, scalar1=float(W - WS), scalar2=None,
                        op0=OP.min)
        gint = pool.tile([NPTS, 2], I32, tag="gint")  # t0, cs
        v.tensor_copy(gint[:], g[:, 1:3])
        gint2 = pool.tile([NPTS // 2, 4], I32, tag="gint2")  # 2 pts per row
        gv = gint[:].rearrange("(a b) j -> a b j", b=2)
        nc.sync.dma_start(gint2[:, 0:2], gv[:, 0, :])
        nc.sync.dma_start(gint2[:, 2:4], gv[:, 1, :])
        v.tensor_copy(csb, gint[:, 1:2])
        v.scalar_tensor_tensor(out=rr0, in0=t0f, scalar=-128.0, in1=r,
                               op0=OP.mult, op1=OP.add)

        # rowsq first: its flattens ride the HWDGE queues early
        rsq = pool.tile([NPTS, 256], F32, tag="rsq")
        v.tensor_scalar(out=rsq[:, 0:128], in0=parange, scalar1=rr0,
                        scalar2=0.2, op0=OP.subtract, op1=OP.mult)
        v.tensor_tensor(out=rsq[:, 0:128], in0=rsq[:, 0:128],
                        in1=rsq[:, 0:128], op=OP.mult)
        v.tensor_scalar(out=rsq[:, 128:256], in0=parange128, scalar1=rr0,
                        scalar2=0.2, op0=OP.subtract, op1=OP.mult)
        v.tensor_tensor(out=rsq[:, 128:256], in0=rsq[:, 128:256],
                        in1=rsq[:, 128:256], op=OP.mult)
        nc.sync.dma_start(
            LL[0:1, :].rearrange("o (i p) -> o i p", i=NPTS),
            rsq[:, 0:128].rearrange("i p -> i () p"))
        nc.scalar.dma_start(
            LL[1:2, :].rearrange("o (i p) -> o i p", i=NPTS),
            rsq[:, 128:256].rearrange("i p -> i () p"))

        v.tensor_tensor(out=bc, in0=csb, in1=c, op=OP.subtract)
        csq = pool.tile([NPTS, 96], F32, tag="csq")
        v.tensor_scalar(out=csq[:], in0=w96, scalar1=bc,
                        scalar2=0.2, op0=OP.add, op1=OP.mult)
        v.tensor_tensor(out=csq[:], in0=csq[:], in1=csq[:], op=OP.mult)
        if use_pen:
            v.tensor_scalar(out=csq[:], in0=csq[:], scalar1=pen,
                            scalar2=None, op0=OP.add)
        nc.gpsimd.dma_start(
            RR[2:3, :].rearrange("o (i j) -> o i j", i=NPTS),
            csq[:].rearrange("i j -> i () j"))
        nc.gpsimd.memset(dmaps[1][:], SQBIG)

        # --- main loop over valid points, paired strips ---
        def quads(idxs):
            return [idxs[k:k + 4] for k in range(0, len(idxs), 4)]

        plan = [(0, pr) for pr in quads(vm0)] + [(1, pr) for pr in quads(vm1)]
        sstr = {}

        def emit_front(pi):
            m, pr = plan[pi]
            n = len(pr)
            strip = psum.tile([128, 384], F32)
            for s, i in enumerate(pr):
                nc.tensor.matmul(strip[:, 96 * s:96 * (s + 1)],
                                 LL[:, 128 * i:128 * (i + 1)],
                                 RR[:, 96 * i:96 * (i + 1)],
                                 start=True, stop=True, skip_group_check=True)
            ss = sb_strips.tile([128, 384], BF16)
            nc.scalar.activation(ss[:, 0:96 * n], strip[:, 0:96 * n], AF.Copy)
            nc.gpsimd.tensor_tensor(out=ss[:, 0:96 * n], in0=ss[:, 0:96 * n],
                                    in1=halfs[:, 0:96 * n], op=OP.pow)
            for s, i in enumerate(pr):
                sstr[i] = (ss, s)

        def emit_min2(i0, i1, d4a, d4b):
            # i1 = i0+1 (same gint2 row); one reg load for both points
            a = i0 // 2
            assert i1 is None or i1 == i0 + 1
            if i1 is None:
                ss, s = sstr[i0]
                with v.register() as rt, v.register() as rc:
                    v.reg_load([rt, rc], gint[i0:i0 + 1, 0:2])
                    t0v = bass.make_scalar_value(rt, min_val=0, max_val=2)
                    csv = bass.make_scalar_value(rc, min_val=0, max_val=W - WS)
                    dslice = d4a[:, bass.ds(t0v, 2), bass.ds(csv, WS)]
                    v.tensor_tensor(
                        out=dslice,
                        in0=ss[:, 96 * s:96 * (s + 1)].rearrange(
                            "p (c w) -> p c w", c=2),
                        in1=dslice, op=OP.min)
                return
            with v.register() as rt0, v.register() as rc0, \
                 v.register() as rt1, v.register() as rc1:
                v.reg_load([rt0, rc0, rt1, rc1], gint2[a:a + 1, 0:4])
                for i, rt, rc, d4 in ((i0, rt0, rc0, d4a), (i1, rt1, rc1, d4b)):
                    ss, s = sstr[i]
                    t0v = bass.make_scalar_value(rt, min_val=0, max_val=2)
                    csv = bass.make_scalar_value(rc, min_val=0, max_val=W - WS)
                    dslice = d4[:, bass.ds(t0v, 2), bass.ds(csv, WS)]
                    v.tensor_tensor(
                        out=dslice,
                        in0=ss[:, 96 * s:96 * (s + 1)].rearrange(
                            "p (c w) -> p c w", c=2),
                        in1=dslice, op=OP.min)

        d4s = [dmaps[m][:].rearrange("p (t w) -> p t w", t=NT) for m in range(2)]
        # group consecutive (i, i+1) same-map points sharing a gint2 row
        steps = []
        for m, vm in ((0, vm0), (1, vm1)):
            k = 0
            while k < len(vm):
                i = vm[k]
                if (k + 1 < len(vm) and vm[k + 1] == i + 1 and i % 2 == 0):
                    steps.append((m, i, i + 1))
                    k += 2
                else:
                    steps.append((m, i, None))
                    k += 1
        # map each step to the quad index that must be emitted first
        qidx = {}
        for pi, (m, pr) in enumerate(plan):
            for i in pr:
                qidx[i] = pi
        npi = 0
        for (m, i0, i1) in steps:
            need = max(qidx[i0], qidx[i1] if i1 is not None else 0) + 1
            want = min(max(need, npi) + 1, len(plan))
            while npi < want:
                emit_front(npi)
                npi += 1
            emit_min2(i0, i1, d4s[m], d4s[m])
            if m == 0 and (i1 or i0) == vm0[-1]:
                for t in range(NT):
                    seg = dmaps[0][:, t * W:(t + 1) * W]
                    ob = out_pool.tile([128, W], F32)
                    nc.scalar.activation(ob[:], seg, AF.Tanh, scale=2.0)
                    nc.sync.dma_start(y[0, t * 128:(t + 1) * 128, :], ob[:])
        for t in range(NT):
            seg = dmaps[1][:, t * W:(t + 1) * W]
            ob = out_pool.tile([128, W], F32)
            nc.scalar.activation(ob[:], seg, AF.Tanh, scale=2.0)
            nc.sync.dma_start(y[1, t * 128:(t + 1) * 128, :], ob[:])

    nc.compile()
    return nc


_CACHE = {}


def _get_built(key):
    if key not in _CACHE:
        vm0, vm1, use_pen = key
        _CACHE[key] = (build(valid=(vm0, vm1), use_pen=use_pen), make_consts())
    return _CACHE[key]


def kernel(x: np.ndarray, coords: np.ndarray) -> np.ndarray:
    from concourse.bass_utils import run_bass_kernel_spmd
    assert x.shape == (B, 3, H, W), x.shape
    assert coords.shape == (B, NPTS, 3), coords.shape
    coords = np.ascontiguousarray(coords, dtype=np.float32)

    val = coords[0, :, :2].max(axis=1) >= 0
    same = all(((coords[b, :, :2].max(axis=1) >= 0) == val).all()
               for b in range(B))
    if same:
        vm0 = tuple(i for i in range(NPM) if val[i])
        vm1 = tuple(i for i in range(NPM, NPTS) if val[i])
        nc, consts = _get_built((vm0, vm1, False))
    else:
        # validity differs across batch: one general build, pen handles
        # invalid points on-device
        vm0 = tuple(range(NPM))
        vm1 = tuple(range(NPM, NPTS))
        nc, consts = _get_built((vm0, vm1, True))

    in_maps = [{"coords": coords[b], **consts} for b in range(B)]
    last_err = None
    for _attempt in range(3):
        try:
            res = run_bass_kernel_spmd(nc, in_maps, list(range(B)))
            break
        except Exception as e:  # device occasionally needs one recovery run
            last_err = e
    else:
        raise last_err
    out = np.stack([res.results[b]["y"] for b in range(B)])
    return out.astype(np.float32)


# revision 3
# speedup vs baseline: 1.0044x; 1.0044x over previous
"""DistMaps production kernel (vG pipeline): valid-point specialization + paired strips + trimmed setup.

Same math as v3 (exact-squares K=3 matmul, bf16 sqrt-space maps, Pool
tt-pow, Act tanh-only tail). New: kernel is built for the host-observed
set of valid points (invalid click slots are skipped entirely); strips
are computed two-per-PSUM-tile so copy and pow run once per pair.
build(valid=(tuple_m0, tuple_m1)) with point indices per map.
"""
from contextlib import ExitStack

import numpy as np

import concourse.bass as bass
import concourse.tile as tile
from concourse import bacc, mybir

F32 = mybir.dt.float32
BF16 = mybir.dt.bfloat16
I32 = mybir.dt.int32
AF = mybir.ActivationFunctionType
OP = mybir.AluOpType

B = 8
H = W = 512
NPTS = 24
NPM = 12
NT = 4
WS = 48
CMARG = WS // 2
BIG = 1.0e6
SQBIG = 1000.0


def make_consts():
    parange = np.tile(np.arange(128, dtype=np.float32)[None, :], (NPTS, 1))
    parange128 = parange + 128.0
    w48 = (np.arange(WS, dtype=np.float32))[None, :]
    w96 = np.tile(np.concatenate([w48, w48], 1), (NPTS, 1)).astype(np.float32)
    cbB = np.concatenate([parange, parange128, w96], axis=1)  # [24, 352]
    i0 = np.tile(np.r_[np.ones(WS, np.float32), np.zeros(WS, np.float32)], NPTS)
    i1 = np.tile(np.r_[np.zeros(WS, np.float32), np.ones(WS, np.float32)], NPTS)
    rrc = np.stack([i0, i1])                      # [2, 2304]
    llones = np.ones((1, NPTS * 128), dtype=np.float32)
    return {"cbB": cbB, "rrc": rrc, "llones": llones}


def build(valid=(tuple(range(12)), tuple(range(12, 24))), use_pen=False):
    vm0, vm1 = [list(v) for v in valid]
    nc = bacc.Bacc("TRN2", target_bir_lowering=False, debug=False, num_devices=B)
    coords = nc.dram_tensor("coords", [NPTS, 3], F32, kind="ExternalInput").ap()
    cbB = nc.dram_tensor("cbB", [NPTS, 352], F32, kind="ExternalInput").ap()
    rrc = nc.dram_tensor("rrc", [2, NPTS * 96], F32, kind="ExternalInput").ap()
    llones = nc.dram_tensor("llones", [1, NPTS * 128], F32,
                            kind="ExternalInput").ap()
    y = nc.dram_tensor("y", [2, H, W], F32, kind="ExternalOutput").ap()

    with tile.TileContext(nc) as tc, ExitStack() as ctx:
        pool = ctx.enter_context(tc.tile_pool(name="sb", bufs=1))
        d_pool = ctx.enter_context(tc.tile_pool(name="dmap", bufs=1))
        psum = ctx.enter_context(tc.tile_pool(name="ps", bufs=5, space="PSUM"))
        psum_w = ctx.enter_context(tc.tile_pool(name="psw", bufs=1, space="PSUM"))
        sb_strips = ctx.enter_context(tc.tile_pool(name="sbs", bufs=4))
        out_pool = ctx.enter_context(tc.tile_pool(name="ob", bufs=8))

        # --- input DMAs ---
        coords_sb = pool.tile([NPTS, 3], F32, tag="coords_sb")
        nc.sync.dma_start(coords_sb[:], coords[:])
        cb = pool.tile([NPTS, 352], F32, tag="cb_sb")
        nc.scalar.dma_start(cb[:], cbB[:])
        parange = cb[:, 0:128]
        parange128 = cb[:, 128:256]
        w96 = cb[:, 256:352]
        RR = pool.tile([3, NPTS * 96], F32, tag="RR")
        nc.scalar.dma_start(RR[0:2, :], rrc[:])
        LL = pool.tile([3, NPTS * 128], F32, tag="LL")
        nc.gpsimd.dma_start(LL[2:3, :], llones[:])

        # --- act table prefetch (Tanh) + pow exponent tile (bf16) ---
        dummy = pool.tile([1, 8], F32, tag="dummy")
        nc.gpsimd.memset(dummy[:], 0.0)
        nc.scalar.activation(dummy[:], dummy[:], AF.Tanh, scale=2.0)
        halfs = pool.tile([128, 384], BF16, tag="halfs")
        nc.gpsimd.memset(halfs[:], 0.5)

        dmaps = []
        for m in range(2):
            dm = d_pool.tile([128, NT * W], BF16, tag=f"d{m}")
            dmaps.append(dm)
        nc.gpsimd.memset(dmaps[0][:], SQBIG)

        # --- PE warmup ---
        wps = psum_w.tile([128, 176], F32, tag="warm")
        for _ in range(6):
            nc.tensor.matmul(wps[:], cb[0:1, 0:128], cb[0:1, 0:176],
                             start=True, stop=True)

        # --- geometry (DVE) ---
        r = coords_sb[:, 0:1]
        c = coords_sb[:, 1:2]
        g = pool.tile([NPTS, 8], F32, tag="geo")
        pen = g[:, 0:1]
        t0f = g[:, 1:2]
        csf = g[:, 2:3]
        csb = g[:, 3:4]
        rr0 = g[:, 4:5]
        bc = g[:, 5:6]
        yrow = g[:, 6:7]

        v = nc.vector
        if use_pen:
            v.tensor_tensor(out=pen, in0=r, in1=c, op=OP.max)
            v.tensor_scalar(out=pen, in0=pen, scalar1=0.0, scalar2=BIG,
                            op0=OP.is_lt, op1=OP.mult)
        v.tensor_scalar(out=yrow, in0=r, scalar1=-23.0, scalar2=1.0 / 128.0,
                        op0=OP.add, op1=OP.mult)
        v.tensor_scalar(out=t0f, in0=yrow, scalar1=1.0, scalar2=None, op0=OP.is_ge)
        v.scalar_tensor_tensor(out=t0f, in0=yrow, scalar=2.0, in1=t0f,
                               op0=OP.is_ge, op1=OP.add)
        v.tensor_scalar(out=csf, in0=c, scalar1=float(-CMARG), scalar2=0.0,
                        op0=OP.add, op1=OP.max)
        v.tensor_scalar(out=csf, in0=csf, scalar1=float(W - WS), scalar2=None,
                        op0=OP.min)
        gint = pool.tile([NPTS, 2], I32, tag="gint")  # t0, cs
        v.tensor_copy(gint[:], g[:, 1:3])
        gint2 = pool.tile([NPTS // 2, 4], I32, tag="gint2")  # 2 pts per row
        gv = gint[:].rearrange("(a b) j -> a b j", b=2)
        nc.sync.dma_start(gint2[:, 0:2], gv[:, 0, :])
        nc.sync.dma_start(gint2[:, 2:4], gv[:, 1, :])
        v.tensor_copy(csb, gint[:, 1:2])
        v.scalar_tensor_tensor(out=rr0, in0=t0f, scalar=-128.0, in1=r,
                               op0=OP.mult, op1=OP.add)

        # colsq first: Pool's SWDGE flatten has the longest pipe
        v.tensor_tensor(out=bc, in0=csb, in1=c, op=OP.subtract)
        csq = pool.tile([NPTS, 96], F32, tag="csq")
        v.tensor_scalar(out=csq[:], in0=w96, scalar1=bc,
                        scalar2=0.2, op0=OP.add, op1=OP.mult)
        v.tensor_tensor(out=csq[:], in0=csq[:], in1=csq[:], op=OP.mult)
        if use_pen:
            v.tensor_scalar(out=csq[:], in0=csq[:], scalar1=pen,
                            scalar2=None, op0=OP.add)
        nc.gpsimd.dma_start(
            RR[2:3, :].rearrange("o (i j) -> o i j", i=NPTS),
            csq[:].rearrange("i j -> i () j"))

        # rowsq both halves fused (cb[:,0:256] = [parange|parange+128])
        rsq = pool.tile([NPTS, 256], F32, tag="rsq")
        v.tensor_scalar(out=rsq[:], in0=cb[:, 0:256], scalar1=rr0,
                        scalar2=0.2, op0=OP.subtract, op1=OP.mult)
        v.tensor_tensor(out=rsq[:], in0=rsq[:], in1=rsq[:], op=OP.mult)
        nc.sync.dma_start(
            LL[0:1, :].rearrange("o (i p) -> o i p", i=NPTS),
            rsq[:, 0:128].rearrange("i p -> i () p"))
        nc.scalar.dma_start(
            LL[1:2, :].rearrange("o (i p) -> o i p", i=NPTS),
            rsq[:, 128:256].rearrange("i p -> i () p"))

        # map1 init on otherwise-idle DVE (Pool must get to pows fast)
        v.memset(dmaps[1][:], SQBIG)

        # --- main loop over valid points, paired strips ---
        def groups(idxs):
            out = [idxs[0:2]]
            k = 2
            while k < len(idxs):
                out.append(idxs[k:k + 4])
                k += 4
            return out

        plan = [(0, pr) for pr in groups(vm0)] + [(1, pr) for pr in groups(vm1)]
        sstr = {}

        def emit_front(pi):
            m, pr = plan[pi]
            n = len(pr)
            strip = psum.tile([128, 384], F32)
            for s, i in enumerate(pr):
                nc.tensor.matmul(strip[:, 96 * s:96 * (s + 1)],
                                 LL[:, 128 * i:128 * (i + 1)],
                                 RR[:, 96 * i:96 * (i + 1)],
                                 start=True, stop=True, skip_group_check=True)
            ss = sb_strips.tile([128, 384], BF16)
            nc.scalar.activation(ss[:, 0:96 * n], strip[:, 0:96 * n], AF.Copy)
            if m == 0 and n == 4:
                nc.gpsimd.tensor_tensor(out=ss[:, 0:192], in0=ss[:, 0:192],
                                        in1=halfs[:, 0:192], op=OP.pow)
                nc.gpsimd.tensor_tensor(out=ss[:, 192:384], in0=ss[:, 192:384],
                                        in1=halfs[:, 192:384], op=OP.pow)
            else:
                nc.gpsimd.tensor_tensor(out=ss[:, 0:96 * n], in0=ss[:, 0:96 * n],
                                        in1=halfs[:, 0:96 * n], op=OP.pow)
            for s, i in enumerate(pr):
                sstr[i] = (ss, s)

        def emit_min2(i0, i1, d4a, d4b):
            # i1 = i0+1 (same gint2 row); one reg load for both points
            a = i0 // 2
            assert i1 is None or i1 == i0 + 1
            if i1 is None:
                ss, s = sstr[i0]
                with v.register() as rt, v.register() as rc:
                    v.reg_load([rt, rc], gint[i0:i0 + 1, 0:2])
                    t0v = bass.make_scalar_value(rt, min_val=0, max_val=2)
                    csv = bass.make_scalar_value(rc, min_val=0, max_val=W - WS)
                    dslice = d4a[:, bass.ds(t0v, 2), bass.ds(csv, WS)]
                    v.tensor_tensor(
                        out=dslice,
                        in0=ss[:, 96 * s:96 * (s + 1)].rearrange(
                            "p (c w) -> p c w", c=2),
                        in1=dslice, op=OP.min)
                return
            with v.register() as rt0, v.register() as rc0, \
                 v.register() as rt1, v.register() as rc1:
                v.reg_load([rt0, rc0, rt1, rc1], gint2[a:a + 1, 0:4])
                for i, rt, rc, d4 in ((i0, rt0, rc0, d4a), (i1, rt1, rc1, d4b)):
                    ss, s = sstr[i]
                    t0v = bass.make_scalar_value(rt, min_val=0, max_val=2)
                    csv = bass.make_scalar_value(rc, min_val=0, max_val=W - WS)
                    dslice = d4[:, bass.ds(t0v, 2), bass.ds(csv, WS)]
                    v.tensor_tensor(
                        out=dslice,
                        in0=ss[:, 96 * s:96 * (s + 1)].rearrange(
                            "p (c w) -> p c w", c=2),
                        in1=dslice, op=OP.min)

        d4s = [dmaps[m][:].rearrange("p (t w) -> p t w", t=NT) for m in range(2)]
        # group consecutive (i, i+1) same-map points sharing a gint2 row
        steps = []
        for m, vm in ((0, vm0), (1, vm1)):
            k = 0
            while k < len(vm):
                i = vm[k]
                if (k + 1 < len(vm) and vm[k + 1] == i + 1 and i % 2 == 0):
                    steps.append((m, i, i + 1))
                    k += 2
                else:
                    steps.append((m, i, None))
                    k += 1
        # map each step to the quad index that must be emitted first
        qidx = {}
        for pi, (m, pr) in enumerate(plan):
            for i in pr:
                qidx[i] = pi
        npi = 0
        for (m, i0, i1) in steps:
            need = max(qidx[i0], qidx[i1] if i1 is not None else 0) + 1
            want = min(max(need, npi) + 1, len(plan))
            while npi < want:
                emit_front(npi)
                npi += 1
            emit_min2(i0, i1, d4s[m], d4s[m])
            if m == 0 and (i1 or i0) == vm0[-1]:
                for t in range(NT):
                    seg = dmaps[0][:, t * W:(t + 1) * W]
                    ob = out_pool.tile([128, W], F32)
                    nc.scalar.activation(ob[:], seg, AF.Tanh, scale=2.0)
                    nc.sync.dma_start(y[0, t * 128:(t + 1) * 128, :], ob[:])
        for t in range(NT):
            seg = dmaps[1][:, t * W:(t + 1) * W]
            ob = out_pool.tile([128, W], F32)
            nc.scalar.activation(ob[:], seg, AF.Tanh, scale=2.0)
            nc.sync.dma_start(y[1, t * 128:(t + 1) * 128, :], ob[:])

    nc.compile()
    return nc

_CACHE = {}


def _get_built(key):
    if key not in _CACHE:
        vm0, vm1, use_pen = key
        _CACHE[key] = (build(valid=(vm0, vm1), use_pen=use_pen), make_consts())
    return _CACHE[key]


def kernel(x: np.ndarray, coords: np.ndarray) -> np.ndarray:
    from concourse.bass_utils import run_bass_kernel_spmd
    assert x.shape == (B, 3, H, W), x.shape
    assert coords.shape == (B, NPTS, 3), coords.shape
    coords = np.ascontiguousarray(coords, dtype=np.float32)

    val = coords[0, :, :2].max(axis=1) >= 0
    same = all(((coords[b, :, :2].max(axis=1) >= 0) == val).all()
               for b in range(B))
    if same:
        vm0 = tuple(i for i in range(NPM) if val[i])
        vm1 = tuple(i for i in range(NPM, NPTS) if val[i])
        nc, consts = _get_built((vm0, vm1, False))
    else:
        # validity differs across batch: one general build, pen handles
        # invalid points on-device
        vm0 = tuple(range(NPM))
        vm1 = tuple(range(NPM, NPTS))
        nc, consts = _get_built((vm0, vm1, True))

    in_maps = [{"coords": coords[b], **consts} for b in range(B)]
    last_err = None
    for _attempt in range(3):
        try:
            res = run_bass_kernel_spmd(nc, in_maps, list(range(B)))
            break
        except Exception as e:  # device occasionally needs one recovery run
            last_err = e
    else:
        raise last_err
    out = np.stack([res.results[b]["y"] for b in range(B)])
    return out.astype(np.float32)
